# revision 9
# baseline (speedup 1.0000x reference)
"""Trainium2 Bass kernel for nn_MixNode (soft decision tree / MoE routing).

The recursive MixNode tree collapses algebraically:
    out[b] = sum_m C_m(x_b) * leafG[m]
where leafG folds the (input-independent) gamma-softmax products into the
leaf vectors, and C_m = prod of routing probabilities along the root->m
path. With delta = logit0 - logit1 per internal node, the two routing
probs are sigmoid(+-delta), so log C = A @ (-softplus(-+delta)) for a
constant 0/-1 path matrix A. softplus(z) = ln(exp(z) + 1) is computed as
Exp then Ln(x + 1); both +-delta blocks are produced by one doubled
matmul (weights [-Wd; +Wd]) so each chunk needs only one Exp + one Ln.

Device pipeline per core (batch shard 2048 rows, 4 chunks of 512):
    DMA x -> PE transpose (feature-major) -> mm1 D2 = [-Wd;+Wd] @ x^T
    -> ACT exp -> ACT ln1p -> sp -> mmA S = A @ sp -> ACT exp -> C
    -> mm2 out = (C-slices)^T @ leafG -> DMA out.
ACT work is ordered in function phases (all Exp, all Ln, all Exp) to
avoid per-op activation-table reloads (~2.7us each).

Sharding: pure data parallelism over the batch dim across 8 cores;
the small tree parameters are folded host-side and replicated.
"""

import os
import sys

import numpy as np

for _p in ("/opt/trn_rl_repo", "/root/.axon_site/_ro/trn_rl_repo"):
    if os.path.isdir(_p) and _p not in sys.path:
        sys.path.append(_p)

import concourse.tile as tile
from concourse import bacc, mybir
from concourse.bass_utils import run_bass_kernel_spmd

N_CORES = 8
BATCH, D_IN, D_OUT = 16384, 512, 128
B_CORE = BATCH // N_CORES  # 2048
N_INT, N_ALL = 31, 63
P = 128
NCH = 4  # batch chunks per core, 512 rows each
SC = 4   # 128-row subtiles per chunk
KC = 4   # 128-feature chunks

F32 = mybir.dt.float32
F32R = mybir.dt.float32r
AF = mybir.ActivationFunctionType

# float32r matmuls run the PE at full rate (vs 4 cycles/row for exact fp32)
# at reduced multiply precision. Overridable for experiments.
MM_FAST = os.environ.get("MIX_MM_FAST", "0") == "1"


def _emit(nc, mm_fast: bool):
    x_d = nc.dram_tensor("x", [B_CORE, D_IN], F32, kind="ExternalInput")
    wdT_d = nc.dram_tensor("wdT", [KC, P, 32], F32, kind="ExternalInput")
    aT_d = nc.dram_tensor("aT", [64, N_ALL], F32, kind="ExternalInput")
    leafG_d = nc.dram_tensor("leafG", [N_ALL, D_OUT], F32, kind="ExternalInput")
    biasN_d = nc.dram_tensor("biasN", [32, 1], F32, kind="ExternalInput")
    ident_d = nc.dram_tensor("ident", [P, P], F32, kind="ExternalInput")
    y_d = nc.dram_tensor("y", [B_CORE, D_OUT], F32, kind="ExternalOutput")

    # float32r matmul inputs must be produced as float32r (HW rounds the
    # mantissa at the producer); constants get a one-time rounding copy.
    mdt = F32R if mm_fast else F32

    with tile.TileContext(nc) as tc:
        with (
            tc.tile_pool(name="const", bufs=1) as constp,
            tc.tile_pool(name="xin", bufs=3) as xinp,
            tc.tile_pool(name="xtp", bufs=2) as xtpp,
            tc.tile_pool(name="act", bufs=4) as actp,
            tc.tile_pool(name="spc", bufs=4) as spp,
            tc.tile_pool(name="ccp", bufs=4) as ccp,
            tc.tile_pool(name="osbp", bufs=2) as osbp,
            tc.tile_pool(name="tps", bufs=2, space="PSUM") as tpsp,
            tc.tile_pool(name="dps", bufs=2, space="PSUM") as dpsp,
            tc.tile_pool(name="sps", bufs=2, space="PSUM") as spsp,
            tc.tile_pool(name="ops", bufs=2, space="PSUM") as opsp,
        ):
            ident = constp.tile([P, P], F32)
            nc.sync.dma_start(ident[:], ident_d[:])
            wdT0 = constp.tile([P, KC, 32], F32)
            for k in range(KC):
                nc.sync.dma_start(wdT0[:, k, :], wdT_d[k])
            aT0 = constp.tile([64, N_ALL], F32)
            nc.sync.dma_start(aT0[:], aT_d[:])
            leafG0 = constp.tile([N_ALL, D_OUT], F32)
            nc.sync.dma_start(leafG0[:], leafG_d[:])
            biasN = constp.tile([32, 1], F32)
            nc.sync.dma_start(biasN[:], biasN_d[:])
            if mm_fast:
                wdT = constp.tile([P, KC, 32], F32R)
                nc.vector.tensor_copy(wdT[:], wdT0[:])
                aT = constp.tile([64, N_ALL], F32R)
                nc.vector.tensor_copy(aT[:], aT0[:])
                leafG = constp.tile([N_ALL, D_OUT], F32R)
                nc.vector.tensor_copy(leafG[:], leafG0[:])
            else:
                wdT, aT, leafG = wdT0, aT0, leafG0

            ts, sp_l, cc_l, acts = [], [], [], []

            # Phase 1: load, transpose, doubled routing matmul, Exp.
            for c in range(NCH):
                xin = xinp.tile([P, SC, D_IN], F32)
                xsrc = x_d[c * 512:(c + 1) * 512, :].rearrange(
                    "(s p) f -> p s f", p=P)
                nc.sync.dma_start(xin[:], xsrc)

                # xT[k] = x-chunk^T (feature-major), [128f, 512b]
                xT = xtpp.tile([P, KC, 512], mdt)
                for k in range(KC):
                    tps = tpsp.tile([P, 512], F32)
                    for s in range(SC):
                        nc.tensor.matmul(
                            tps[:, s * P:(s + 1) * P],
                            xin[:, s, k * P:(k + 1) * P],
                            ident[:],
                            is_transpose=True,
                            start=(s == 0),
                            stop=(s == SC - 1),
                        )
                    nc.vector.tensor_copy(xT[:, k, :], tps[:])

                # mm1: D = Wd @ x^T [32, 512], row 31 exactly 0 (padded)
                dps = dpsp.tile([32, 512], F32)
                for k in range(KC):
                    nc.tensor.matmul(
                        dps[:],
                        wdT[:, k, :],
                        xT[:, k, :],
                        start=(k == 0),
                        stop=(k == KC - 1),
                    )
                t = actp.tile([64, 512], F32, tag="texp")
                acts.append(nc.scalar.activation(
                    t[0:32, :], dps[:], AF.Exp, bias=biasN[:], scale=-1.0))
                nc.vector.reciprocal(t[32:64, :], t[0:32, :])
                ts.append(t)

            # Phase 2: Ln(x+1) -> softplus blocks (pad rows give ln2,
            # nulled by the zero rows of A).
            for c in range(NCH):
                sp = spp.tile([64, 512], mdt, tag="sp")
                acts.append(nc.scalar.activation(sp[:], ts[c][:], AF.Ln, bias=1.0))
                sp_l.append(sp)

            # Phase 3: S = A @ sp, C = exp(S).
            for c in range(NCH):
                sps = spsp.tile([N_ALL, 512], F32)
                nc.tensor.matmul(sps[:], aT[:], sp_l[c][:], start=True, stop=True)
                cc = ccp.tile([N_ALL, 512], mdt, tag="cc")
                acts.append(nc.scalar.activation(cc[:], sps[:], AF.Exp))
                cc_l.append(cc)

            # Phase 4: out = C^T @ leafG, store.
            for c in range(NCH):
                cc = cc_l[c]
                ops = opsp.tile([P, 512], F32)
                for s in range(SC):
                    nc.tensor.matmul(
                        ops[:, s * P:(s + 1) * P],
                        cc[:, s * P:(s + 1) * P],
                        leafG[:],
                        start=(s == 0),
                        stop=(s == SC - 1),
                    )
                osb = osbp.tile([P, SC, D_OUT], F32)
                acts.append(nc.scalar.copy(
                    osb[:].rearrange("p s o -> p (s o)"), ops[:]))
                ydst = y_d[c * 512:(c + 1) * 512, :].rearrange(
                    "(s p) o -> p s o", p=P)
                nc.sync.dma_start(ydst, osb[:])

            from concourse.tile_rust import add_dep_helper
            for prev, nxt in zip(acts, acts[1:]):
                add_dep_helper(nxt.ins, prev.ins, sync=False,
                               reason="ACT table-set ordering")
    return nc


_BUILD_CACHE = {}


def build(mm_fast: bool = MM_FAST):
    key = bool(mm_fast)
    if key not in _BUILD_CACHE:
        nc = bacc.Bacc("TRN2", target_bir_lowering=False, debug=False,
                       num_devices=N_CORES)
        _emit(nc, mm_fast)
        nc.compile()
        _BUILD_CACHE[key] = nc
    return _BUILD_CACHE[key]


def host_prep(W, b, gamma, leaf):
    """Fold the tiny tree parameters into the kernel's constant tensors."""
    W = np.asarray(W, np.float32)
    b = np.asarray(b, np.float32)
    gamma = np.asarray(gamma, np.float32)
    leaf = np.asarray(leaf, np.float32)

    Wd = W[:, 0, :] - W[:, 1, :]                      # [31, 512]
    bd = b[:, 0] - b[:, 1]                            # [31]
    e = np.exp(gamma - gamma.max(-1, keepdims=True))
    g = e / e.sum(-1, keepdims=True)                  # [31, 2]

    path = np.zeros(N_ALL, np.float64)
    path[0] = 1.0
    for m in range(1, N_ALL):
        par = (m - 1) // 2
        path[m] = path[par] * g[par, 0]
    G = np.array([path[m] * (g[m, 1] if m < N_INT else 1.0)
                  for m in range(N_ALL)])
    leafG = (G[:, None] * leaf.astype(np.float64)).astype(np.float32)

    # A[row, m] = -1 if the edge lives on the root->m path.
    # Edge (node a, child j) -> row a (j=0) or row 32+a (j=1); rows 31/63 pad.
    A = np.zeros((64, N_ALL), np.float32)
    for m in range(N_ALL):
        node = m
        while node:
            par = (node - 1) // 2
            j = node - 2 * par - 1
            A[par if j == 0 else 32 + par, m] = -1.0
            node = par

    wdT = np.zeros((KC, P, 32), np.float32)
    wdTfull = np.ascontiguousarray(Wd.T)              # [512, 31]
    for k in range(KC):
        wdT[k, :, 0:N_INT] = wdTfull[k * P:(k + 1) * P]

    biasN = np.zeros((32, 1), np.float32)
    biasN[0:N_INT, 0] = -bd
    ident = np.eye(P, dtype=np.float32)
    return {
        "wdT": wdT,
        "aT": np.ascontiguousarray(A),
        "leafG": np.ascontiguousarray(leafG),
        "biasN": biasN,
        "ident": ident,
    }


def run(x, W, b, gamma, leaf, mm_fast: bool = MM_FAST, **spmd_kwargs):
    x = np.asarray(x, np.float32)
    consts = host_prep(W, b, gamma, leaf)
    shards = x.reshape(N_CORES, B_CORE, D_IN)
    in_maps = [dict(consts, x=np.ascontiguousarray(shards[i]))
               for i in range(N_CORES)]
    nc = build(mm_fast)
    res = run_bass_kernel_spmd(nc, in_maps, list(range(N_CORES)), **spmd_kwargs)
    y = np.concatenate([res.results[i]["y"] for i in range(N_CORES)], axis=0)
    return y, res


def kernel(x, W, b, gamma, leaf):
    y, _ = run(x, W, b, gamma, leaf)
    return y
